# revision 19
# baseline (speedup 1.0000x reference)
"""TRN2 Bass/Tile kernel: BatchNorm1d + 4-head self-attention + out-projection.

Reference computation (b=4, c=256, n=4096, heads=4, d=64):
    xn   = BN(x)  (training-mode stats over batch+length)
    qkv  = w_qkv @ xn ;  q,k,v  (q scaled by d^-0.5)
    out  = softmax(q^T k) @ v^T  per (batch, head)
    y    = w_out @ out + b_out

Sharding over 8 NeuronCores: core i handles (batch i//2, query-half i%2).
Each core receives x[b] split into [my half | other half] so it can compute
BN partial stats over exactly its own (b, half) slice; a [256,2] AllReduce
produces exact global BN stats. Keys/values are processed in the core-local
order [mine, other] (softmax + attention are invariant to key permutation).

Attention is computed in the transposed-score layout S^T[key, query] so that
no PE transposes are needed anywhere:
  - scores:  lhsT = k-block [d,128key], rhs = q [d,512q]  -> S^T in PSUM
  - exp on ACT from PSUM in groups of 3 key-chunks (amortize ACT overhead)
  - AV:      lhsT = vT-block [128key, 65], rhs = exp(S^T)  (the 65th column
             of vT is ones, producing the softmax denominator for free)
  - per-query 1/denominator broadcast via gpsimd partition_broadcast
All matmuls use float32r (full-rate fp32 on the PE for moving dim >= 256).
"""

import numpy as np

import concourse.bacc as bacc
import concourse.tile as tile
from concourse import mybir
from concourse.bass_utils import run_bass_kernel_spmd

B, C, N = 4, 256, 4096
H, D = 4, 64
P = 128
CT = C // P            # 2 channel tiles of 128
RB = 2                 # row blocks for q/k rows (256 = 2*128)
NH = N // 2            # 2048 queries per core
QS = 512               # query subtile (1 PSUM bank of fp32)
NQS = NH // QS         # 4
KC = 128               # key chunk (matmul stationary width)
NKC = N // KC          # 32
G = 3                  # key chunks per exp group (3 PSUM banks)
NG = (NKC + G - 1) // G
EPS = 1e-5
SCALE = D ** -0.5
F32 = mybir.dt.float32
F32R = mybir.dt.float32r
MMDT = F32R  # dtype for every tensor feeding the PE (TF32 on HW)
NCORES = 8

# Set to False to replace gpsimd.partition_broadcast with a PE ones-matmul.
USE_GPSIMD_BCAST = True
# When True, run the attention matmuls (scores + AV) in bf16 instead of f32r.
ATTN_BF16 = False
# Moving size of the keep-warm dummy matmuls (0 disables them).
DUMMY_N = 0


def _body(tc, x_mine, x_other, w_qkvT, w_outT, bn_w, bn_b, b_out, out):
    from contextlib import ExitStack

    nc = tc.nc
    AF = mybir.ActivationFunctionType
    OP = mybir.AluOpType

    with ExitStack() as ctx:
        big = ctx.enter_context(tc.tile_pool(name="big", bufs=1))
        small = ctx.enter_context(tc.tile_pool(name="small", bufs=1))
        epool = ctx.enter_context(tc.tile_pool(name="epool", bufs=4))
        rpool = ctx.enter_context(tc.tile_pool(name="rpool", bufs=2))
        opool = ctx.enter_context(tc.tile_pool(name="opool", bufs=2))
        spool = ctx.enter_context(tc.tile_pool(name="spool", bufs=2, space="PSUM"))
        mmpool = ctx.enter_context(tc.tile_pool(name="mmpool", bufs=2, space="PSUM"))
        dram = ctx.enter_context(tc.tile_pool(name="dram", bufs=1, space="DRAM"))

        # ---- loads ------------------------------------------------------
        xn_sb = big.tile([P, CT, N], MMDT, tag="xnattn")  # key order: [mine | other]
        xm_r = x_mine.rearrange("(ct p) n -> p ct n", p=P)
        for ct in range(CT):
            for half in range(2):
                nc.sync.dma_start(
                    out=xn_sb[:, ct, half * (NH // 2) : (half + 1) * (NH // 2)],
                    in_=xm_r[:, ct, half * (NH // 2) : (half + 1) * (NH // 2)],
                )
        nc.sync.dma_start(
            out=xn_sb[:, :, NH:N], in_=x_other.rearrange("(ct p) n -> p ct n", p=P)
        )
        wq_sb = big.tile([P, CT, 3 * C], MMDT)
        nc.sync.dma_start(
            out=wq_sb, in_=w_qkvT.rearrange("(ct p) o -> p ct o", p=P)
        )
        wo_sb = big.tile([P, CT, C], MMDT)
        nc.sync.dma_start(out=wo_sb, in_=w_outT.rearrange("(ct p) o -> p ct o", p=P))
        bnw_sb = small.tile([P, CT, 1], F32)
        nc.sync.dma_start(out=bnw_sb, in_=bn_w)
        bnb_sb = small.tile([P, CT, 1], F32)
        nc.sync.dma_start(out=bnb_sb, in_=bn_b)
        bo_sb = small.tile([P, CT, 1], F32)
        nc.sync.dma_start(out=bo_sb, in_=b_out)

        # ---- BN stats over my (batch, half) slice + AllReduce -----------
        SG = NH // 512  # bn_stats subgroups
        stat6 = small.tile([P, CT, SG, 6], F32)
        for ct in range(CT):
            xm = xn_sb[:, ct, 0:NH].rearrange("p (s f) -> p s f", f=512)
            for s in range(SG):
                nc.vector.bn_stats(out=stat6[:, ct, s, :], in_=xm[:, s, :])
        mv = small.tile([P, CT, 2], F32)
        for ct in range(CT):
            nc.vector.bn_aggr(out=mv[:, ct, :], in_=stat6[:, ct])

        # pack [mean, E[x^2]] = [mean, var + mean^2]
        ccin_sb = small.tile([P, CT, 2], F32)
        nc.vector.tensor_copy(out=ccin_sb[:, :, 0:1], in_=mv[:, :, 0:1])
        msq = small.tile([P, CT, 1], F32)
        nc.vector.tensor_mul(out=msq, in0=mv[:, :, 0:1], in1=mv[:, :, 0:1])
        nc.vector.tensor_add(out=ccin_sb[:, :, 1:2], in0=mv[:, :, 1:2], in1=msq)

        cc_in = dram.tile([C, 2], F32)
        cc_out = dram.tile([C, 2], F32, addr_space="Shared")
        nc.sync.dma_start(
            out=cc_in.rearrange("(ct p) s -> p ct s", p=P), in_=ccin_sb
        )
        nc.gpsimd.collective_compute(
            "AllReduce",
            OP.add,
            replica_groups=[list(range(NCORES))],
            ins=[cc_in.opt()],
            outs=[cc_out.opt()],
        )
        gstat = small.tile([P, CT, 2], F32)
        nc.sync.dma_start(
            out=gstat, in_=cc_out.rearrange("(ct p) s -> p ct s", p=P)
        )

        # global mean/var -> scale s = bn_w * rstd, shift = bn_b - mean * s
        mean_g = small.tile([P, CT, 1], F32)
        nc.vector.tensor_scalar_mul(
            out=mean_g, in0=gstat[:, :, 0:1], scalar1=1.0 / NCORES
        )
        e2 = small.tile([P, CT, 1], F32)
        nc.vector.tensor_scalar_mul(
            out=e2, in0=gstat[:, :, 1:2], scalar1=1.0 / NCORES
        )
        var_g = small.tile([P, CT, 1], F32)
        nc.vector.tensor_mul(out=var_g, in0=mean_g, in1=mean_g)
        nc.vector.tensor_sub(out=var_g, in0=e2, in1=var_g)
        eps_sb = small.tile([P, 1], F32)
        nc.vector.memset(eps_sb, EPS)
        sd = small.tile([P, CT, 1], F32)
        nc.scalar.activation(out=sd, in_=var_g, func=AF.Sqrt, bias=eps_sb)
        rstd = small.tile([P, CT, 1], F32)
        nc.vector.reciprocal(out=rstd, in_=sd)
        s_sb = small.tile([P, CT, 1], F32)
        nc.vector.tensor_mul(out=s_sb, in0=bnw_sb, in1=rstd)
        shift_sb = small.tile([P, CT, 1], F32)
        nc.vector.tensor_mul(out=shift_sb, in0=mean_g, in1=s_sb)
        nc.vector.tensor_sub(out=shift_sb, in0=bnb_sb, in1=shift_sb)

        # xn = x * s + shift (in place)
        for ct in range(CT):
            nc.vector.tensor_scalar(
                out=xn_sb[:, ct, :],
                in0=xn_sb[:, ct, :],
                scalar1=s_sb[:, ct],
                scalar2=shift_sb[:, ct],
                op0=OP.mult,
                op1=OP.add,
            )

        # fold q scaling d^-0.5 into the q columns of w_qkvT
        nc.vector.tensor_scalar_mul(
            out=wq_sb[:, :, 0:C], in0=wq_sb[:, :, 0:C], scalar1=SCALE
        )

        # ---- QKV projections -------------------------------------------
        # q/k are stored zero-padded to 128 partitions per head (rows 64:128
        # are zeros): a K=64 matmul streams its operands at HALF the SBUF
        # bandwidth, padding the contraction to 128 restores full rate.
        # vT is padded to 128 columns per head (cols 65:128 zero) for the same
        # reason on the AV output partitions, and kept in bf16 together with
        # exp(S) so the AV matmul gets fast-weight-load and half the SBUF.
        BF16 = mybir.dt.bfloat16
        ADT = BF16 if ATTN_BF16 else MMDT
        q_pad = big.tile([P, H, NH], MMDT)
        k_pad = big.tile([P, H, N], MMDT)
        vT_pad = big.tile([P, NKC, H, P], BF16)
        # all constant-init writes go to the otherwise-idle GPSIMD engine so
        # the DVE can run the BN stats (critical path to the AllReduce) first
        nc.gpsimd.memset(vT_pad[:, :, :, D:P], 0.0)
        nc.gpsimd.memset(vT_pad[:, :, :, D : D + 1], 1.0)
        for h in range(H):
            nc.gpsimd.memset(q_pad[D:P, h, :].bitcast(F32), 0.0)
            nc.gpsimd.memset(k_pad[D:P, h, :].bitcast(F32), 0.0)
        ones_blk = small.tile([P, NKC * H], F32)
        nc.vector.memset(ones_blk, 1.0)

        for rb in range(RB):  # q (my query half only)
            for j in range(NQS):
                ps = mmpool.tile([P, QS], F32, tag="mm")
                for ct in range(CT):
                    nc.tensor.matmul(
                        out=ps,
                        lhsT=wq_sb[:, ct, rb * P : (rb + 1) * P],
                        rhs=xn_sb[:, ct, j * QS : (j + 1) * QS],
                        start=(ct == 0),
                        stop=(ct == CT - 1),
                    )
                nc.vector.tensor_copy(
                    out=q_pad[0:D, 2 * rb, j * QS : (j + 1) * QS], in_=ps[0:D, :]
                )
                nc.vector.tensor_copy(
                    out=q_pad[0:D, 2 * rb + 1, j * QS : (j + 1) * QS],
                    in_=ps[D:P, :],
                )
        for rb in range(RB):  # k (full length)
            for j in range(N // QS):
                ps = mmpool.tile([P, QS], F32, tag="mm")
                for ct in range(CT):
                    nc.tensor.matmul(
                        out=ps,
                        lhsT=wq_sb[:, ct, C + rb * P : C + (rb + 1) * P],
                        rhs=xn_sb[:, ct, j * QS : (j + 1) * QS],
                        start=(ct == 0),
                        stop=(ct == CT - 1),
                    )
                nc.vector.tensor_copy(
                    out=k_pad[0:D, 2 * rb, j * QS : (j + 1) * QS], in_=ps[0:D, :]
                )
                nc.vector.tensor_copy(
                    out=k_pad[0:D, 2 * rb + 1, j * QS : (j + 1) * QS],
                    in_=ps[D:P, :],
                )
        for nb in range(NKC):  # v, produced transposed: [key, (head, d)]
            ps = mmpool.tile([P, C], F32, tag="mm")
            for ct in range(CT):
                nc.tensor.matmul(
                    out=ps,
                    lhsT=xn_sb[:, ct, nb * KC : (nb + 1) * KC],
                    rhs=wq_sb[:, ct, 2 * C : 3 * C],
                    start=(ct == 0),
                    stop=(ct == CT - 1),
                )
            nc.vector.tensor_copy(
                out=vT_pad[:, nb, :, 0:D],
                in_=ps.rearrange("p (h d) -> p h d", d=D),
            )

        # ---- attention --------------------------------------------------
        if not USE_GPSIMD_BCAST:
            ones_sb = small.tile([1, D], MMDT)
            nc.vector.tensor_copy(out=ones_sb, in_=ones_blk[0:1, 0:D])
        if DUMMY_N:
            # tiny "keep-warm" operands: dummy matmuls issued right before the
            # stall points of the PE stream keep the HAM activity monitor from
            # re-throttling the PE clock to 1.2 GHz while ACT catches up.
            dum_sb = small.tile([1, DUMMY_N], ADT)
            nc.vector.tensor_copy(out=dum_sb, in_=ones_blk[0:1, 0:DUMMY_N])

        def dummy_mm():
            if DUMMY_N:
                scrap = mmpool.tile([P, QS], F32, tag="mm", name="scrap")
                nc.tensor.matmul(
                    out=scrap[0:1, 0:DUMMY_N],
                    lhsT=dum_sb[0:1, 0:1],
                    rhs=dum_sb[0:1, 0:DUMMY_N],
                    start=True,
                    stop=True,
                )

        attn_sb = big.tile([P, CT, NH], MMDT, tag="xnattn")
        out_r = out.rearrange("(rb p) n -> p rb n", p=P)
        for j in range(NQS):
            for h in range(H):
                kp = (h % 2) * D
                krb = h // 2
                avp = mmpool.tile([P, QS], F32, tag="mm")
                for g in range(NG):
                    gs = min(G, NKC - g * G)
                    dummy_mm()
                    sp = spool.tile([P, G, QS], F32, tag="sp")
                    for u in range(gs):
                        kc = g * G + u
                        nc.tensor.matmul(
                            out=sp[:, u, :],
                            lhsT=k_pad[:, h, kc * KC : (kc + 1) * KC],
                            rhs=q_pad[:, h, j * QS : (j + 1) * QS],
                            start=True,
                            stop=True,
                        )
                    e_sb = epool.tile([P, G, QS], BF16, tag="e")
                    nc.scalar.activation(
                        out=e_sb[:, 0:gs, :], in_=sp[:, 0:gs, :], func=AF.Exp
                    )
                    dummy_mm()
                    for u in range(gs):
                        kc = g * G + u
                        nc.tensor.matmul(
                            out=avp,
                            lhsT=vT_pad[:, kc, h, :],
                            rhs=e_sb[:, u, :],
                            start=(kc == 0),
                            stop=(kc == NKC - 1),
                        )
                # normalize: rows 0:D divided by the denominator in row D
                r_sb = rpool.tile([1, QS], F32, tag="r")
                nc.vector.reciprocal(out=r_sb, in_=avp[D : D + 1, :])
                rbc = rpool.tile([D, QS], F32, tag="rbc")
                if USE_GPSIMD_BCAST:
                    nc.gpsimd.partition_broadcast(rbc, r_sb)
                else:
                    rbp = mmpool.tile([D, QS], F32, tag="mm")
                    nc.tensor.matmul(
                        out=rbp, lhsT=ones_sb, rhs=r_sb, start=True, stop=True
                    )
                    nc.vector.tensor_copy(out=rbc, in_=rbp)
                nc.vector.tensor_tensor(
                    out=attn_sb[kp : kp + D, krb, j * QS : (j + 1) * QS],
                    in0=avp[0:D, :],
                    in1=rbc,
                    op=OP.mult,
                )

            # output projection + bias for this query block (all heads done)
            for rb in range(RB):
                ps = mmpool.tile([P, QS], F32, tag="mm")
                for ct in range(CT):
                    nc.tensor.matmul(
                        out=ps,
                        lhsT=wo_sb[:, ct, rb * P : (rb + 1) * P],
                        rhs=attn_sb[:, ct, j * QS : (j + 1) * QS],
                        start=(ct == 0),
                        stop=(ct == CT - 1),
                    )
                o_t = opool.tile([P, QS], F32, tag="o")
                nc.vector.tensor_scalar_add(out=o_t, in0=ps, scalar1=bo_sb[:, rb])
                nc.sync.dma_start(
                    out=out_r[:, rb, j * QS : (j + 1) * QS], in_=o_t
                )


def build():
    nc = bacc.Bacc(
        "TRN2", target_bir_lowering=False, debug=False, num_devices=NCORES
    )
    x_mine = nc.dram_tensor("x_mine", [C, NH], MMDT, kind="ExternalInput").ap()
    x_other = nc.dram_tensor("x_other", [C, NH], MMDT, kind="ExternalInput").ap()
    w_qkvT = nc.dram_tensor("w_qkvT", [C, 3 * C], MMDT, kind="ExternalInput").ap()
    w_outT = nc.dram_tensor("w_outT", [C, C], MMDT, kind="ExternalInput").ap()
    bn_w = nc.dram_tensor("bn_w", [P, CT, 1], F32, kind="ExternalInput").ap()
    bn_b = nc.dram_tensor("bn_b", [P, CT, 1], F32, kind="ExternalInput").ap()
    b_out = nc.dram_tensor("b_out", [P, CT, 1], F32, kind="ExternalInput").ap()
    out = nc.dram_tensor("out", [C, NH], F32, kind="ExternalOutput").ap()
    with tile.TileContext(nc) as tc:
        _body(tc, x_mine, x_other, w_qkvT, w_outT, bn_w, bn_b, b_out, out)
    nc.compile()
    return nc


_nc_cache = None


def make_in_maps(x, bn_weight, bn_bias, w_qkv, w_out, b_out):
    x = np.ascontiguousarray(np.asarray(x, dtype=np.float32))
    wqT = np.ascontiguousarray(np.asarray(w_qkv, dtype=np.float32).T)
    woT = np.ascontiguousarray(np.asarray(w_out, dtype=np.float32).T)

    def vec_layout(v):
        v = np.asarray(v, dtype=np.float32)
        return np.ascontiguousarray(v.reshape(CT, P).T.reshape(P, CT, 1))

    bnw = vec_layout(bn_weight)
    bnb = vec_layout(bn_bias)
    bo = vec_layout(b_out)
    in_maps = []
    for core in range(NCORES):
        bi, half = divmod(core, 2)
        mine = np.ascontiguousarray(x[bi][:, half * NH : (half + 1) * NH])
        other = np.ascontiguousarray(x[bi][:, (1 - half) * NH : (2 - half) * NH])
        in_maps.append(
            {
                "x_mine": mine,
                "x_other": other,
                "w_qkvT": wqT,
                "w_outT": woT,
                "bn_w": bnw,
                "bn_b": bnb,
                "b_out": bo,
            }
        )
    return in_maps


def assemble(results):
    outp = np.empty((B, C, N), np.float32)
    for core in range(NCORES):
        bi, half = divmod(core, 2)
        outp[bi][:, half * NH : (half + 1) * NH] = results[core]["out"]
    return outp


def kernel(x, bn_weight, bn_bias, w_qkv, w_out, b_out):
    global _nc_cache
    if _nc_cache is None:
        _nc_cache = build()
    in_maps = make_in_maps(x, bn_weight, bn_bias, w_qkv, w_out, b_out)
    res = run_bass_kernel_spmd(_nc_cache, in_maps, list(range(NCORES)))
    return assemble(res.results)


if __name__ == "__main__":
    rng = np.random.default_rng(0)
    x = rng.standard_normal((B, C, N), dtype=np.float32)
    w_qkv = rng.standard_normal((3 * C, C), dtype=np.float32) * C**-0.5
    w_out = rng.standard_normal((C, C), dtype=np.float32) * C**-0.5
    y = kernel(
        x,
        np.ones(C, np.float32),
        np.zeros(C, np.float32),
        w_qkv,
        w_out,
        np.zeros(C, np.float32),
    )
    print(y.shape, np.abs(y).max())


# revision 21
# speedup vs baseline: 1.0466x; 1.0466x over previous
"""TRN2 Bass/Tile kernel: BatchNorm1d + 4-head self-attention + out-projection.

Reference computation (b=4, c=256, n=4096, heads=4, d=64):
    xn   = BN(x)  (training-mode stats over batch+length)
    qkv  = w_qkv @ xn ;  q,k,v  (q scaled by d^-0.5)
    out  = softmax(q^T k) @ v^T  per (batch, head)
    y    = w_out @ out + b_out

Sharding over 8 NeuronCores: core i handles (batch i//2, query-half i%2).
Each core receives x[b] split into [my half | other half] so it can compute
BN partial stats over exactly its own (b, half) slice; a [256,2] AllReduce
produces exact global BN stats. Keys/values are processed in the core-local
order [mine, other] (softmax + attention are invariant to key permutation).

Attention is computed in the transposed-score layout S^T[key, query] so that
no PE transposes are needed anywhere:
  - scores:  lhsT = k-block [d,128key], rhs = q [d,512q]  -> S^T in PSUM
  - exp on ACT from PSUM in groups of 3 key-chunks (amortize ACT overhead)
  - AV:      lhsT = vT-block [128key, 65], rhs = exp(S^T)  (the 65th column
             of vT is ones, producing the softmax denominator for free)
  - per-query 1/denominator broadcast via gpsimd partition_broadcast
All matmuls use float32r (full-rate fp32 on the PE for moving dim >= 256).
"""

import numpy as np

import concourse.bacc as bacc
import concourse.tile as tile
from concourse import mybir
from concourse.bass_utils import run_bass_kernel_spmd

B, C, N = 4, 256, 4096
H, D = 4, 64
P = 128
CT = C // P            # 2 channel tiles of 128
RB = 2                 # row blocks for q/k rows (256 = 2*128)
NH = N // 2            # 2048 queries per core
QS = 512               # query subtile (1 PSUM bank of fp32)
NQS = NH // QS         # 4
KC = 128               # key chunk (matmul stationary width)
NKC = N // KC          # 32
G = 3                  # key chunks per exp group (3 PSUM banks)
NG = (NKC + G - 1) // G
EPS = 1e-5
SCALE = D ** -0.5
F32 = mybir.dt.float32
F32R = mybir.dt.float32r
MMDT = F32R  # dtype for every tensor feeding the PE (TF32 on HW)
NCORES = 8

# Set to False to replace gpsimd.partition_broadcast with a PE ones-matmul.
USE_GPSIMD_BCAST = True
# When True, run the attention matmuls (scores + AV) in bf16 instead of f32r.
ATTN_BF16 = False
# Moving size of the keep-warm dummy matmuls (0 disables them).
DUMMY_N = 0


def _body(tc, x_mine, x_other, w_qkvT, w_outT, bn_w, bn_b, b_out, out):
    from contextlib import ExitStack

    nc = tc.nc
    AF = mybir.ActivationFunctionType
    OP = mybir.AluOpType

    with ExitStack() as ctx:
        big = ctx.enter_context(tc.tile_pool(name="big", bufs=1))
        small = ctx.enter_context(tc.tile_pool(name="small", bufs=1))
        epool = ctx.enter_context(tc.tile_pool(name="epool", bufs=4))
        rpool = ctx.enter_context(tc.tile_pool(name="rpool", bufs=2))
        opool = ctx.enter_context(tc.tile_pool(name="opool", bufs=2))
        spool = ctx.enter_context(tc.tile_pool(name="spool", bufs=2, space="PSUM"))
        mmpool = ctx.enter_context(tc.tile_pool(name="mmpool", bufs=2, space="PSUM"))
        dram = ctx.enter_context(tc.tile_pool(name="dram", bufs=1, space="DRAM"))

        # ---- loads ------------------------------------------------------
        xn_sb = big.tile([P, CT, N], MMDT, tag="xnattn")  # key order: [mine | other]
        xm_r = x_mine.rearrange("(ct p) n -> p ct n", p=P)
        for ct in range(CT):
            for half in range(2):
                nc.sync.dma_start(
                    out=xn_sb[:, ct, half * (NH // 2) : (half + 1) * (NH // 2)],
                    in_=xm_r[:, ct, half * (NH // 2) : (half + 1) * (NH // 2)],
                )
        nc.sync.dma_start(
            out=xn_sb[:, :, NH:N], in_=x_other.rearrange("(ct p) n -> p ct n", p=P)
        )
        wq_sb = big.tile([P, CT, 3 * C], MMDT)
        nc.sync.dma_start(
            out=wq_sb, in_=w_qkvT.rearrange("(ct p) o -> p ct o", p=P)
        )
        wo_sb = big.tile([P, CT, C], MMDT)
        nc.sync.dma_start(out=wo_sb, in_=w_outT.rearrange("(ct p) o -> p ct o", p=P))
        bnw_sb = small.tile([P, CT, 1], F32)
        nc.sync.dma_start(out=bnw_sb, in_=bn_w)
        bnb_sb = small.tile([P, CT, 1], F32)
        nc.sync.dma_start(out=bnb_sb, in_=bn_b)
        bo_sb = small.tile([P, CT, 1], F32)
        nc.sync.dma_start(out=bo_sb, in_=b_out)

        # ---- BN stats over my (batch, half) slice + AllReduce -----------
        SG = NH // 512  # bn_stats subgroups
        stat6 = small.tile([P, CT, SG, 6], F32)
        for ct in range(CT):
            xm = xn_sb[:, ct, 0:NH].rearrange("p (s f) -> p s f", f=512)
            for s in range(SG):
                nc.vector.bn_stats(out=stat6[:, ct, s, :], in_=xm[:, s, :])
        mv = small.tile([P, CT, 2], F32)
        for ct in range(CT):
            nc.vector.bn_aggr(out=mv[:, ct, :], in_=stat6[:, ct])

        # pack [mean, E[x^2]] = [mean, var + mean^2]
        ccin_sb = small.tile([P, CT, 2], F32)
        nc.vector.tensor_copy(out=ccin_sb[:, :, 0:1], in_=mv[:, :, 0:1])
        msq = small.tile([P, CT, 1], F32)
        nc.vector.tensor_mul(out=msq, in0=mv[:, :, 0:1], in1=mv[:, :, 0:1])
        nc.vector.tensor_add(out=ccin_sb[:, :, 1:2], in0=mv[:, :, 1:2], in1=msq)

        cc_in = dram.tile([C, 2], F32)
        cc_out = dram.tile([C, 2], F32, addr_space="Shared")
        nc.sync.dma_start(
            out=cc_in.rearrange("(ct p) s -> p ct s", p=P), in_=ccin_sb
        )
        nc.gpsimd.collective_compute(
            "AllReduce",
            OP.add,
            replica_groups=[list(range(NCORES))],
            ins=[cc_in.opt()],
            outs=[cc_out.opt()],
        )
        gstat = small.tile([P, CT, 2], F32)
        nc.sync.dma_start(
            out=gstat, in_=cc_out.rearrange("(ct p) s -> p ct s", p=P)
        )

        # global mean/var -> scale s = bn_w * rstd, shift = bn_b - mean * s
        mean_g = small.tile([P, CT, 1], F32)
        nc.vector.tensor_scalar_mul(
            out=mean_g, in0=gstat[:, :, 0:1], scalar1=1.0 / NCORES
        )
        e2 = small.tile([P, CT, 1], F32)
        nc.vector.tensor_scalar_mul(
            out=e2, in0=gstat[:, :, 1:2], scalar1=1.0 / NCORES
        )
        var_g = small.tile([P, CT, 1], F32)
        nc.vector.tensor_mul(out=var_g, in0=mean_g, in1=mean_g)
        nc.vector.tensor_sub(out=var_g, in0=e2, in1=var_g)
        eps_sb = small.tile([P, 1], F32)
        nc.vector.memset(eps_sb, EPS)
        sd = small.tile([P, CT, 1], F32)
        nc.scalar.activation(out=sd, in_=var_g, func=AF.Sqrt, bias=eps_sb)
        rstd = small.tile([P, CT, 1], F32)
        nc.vector.reciprocal(out=rstd, in_=sd)
        s_sb = small.tile([P, CT, 1], F32)
        nc.vector.tensor_mul(out=s_sb, in0=bnw_sb, in1=rstd)
        shift_sb = small.tile([P, CT, 1], F32)
        nc.vector.tensor_mul(out=shift_sb, in0=mean_g, in1=s_sb)
        nc.vector.tensor_sub(out=shift_sb, in0=bnb_sb, in1=shift_sb)

        # xn = x * s + shift (in place)
        for ct in range(CT):
            nc.vector.tensor_scalar(
                out=xn_sb[:, ct, :],
                in0=xn_sb[:, ct, :],
                scalar1=s_sb[:, ct],
                scalar2=shift_sb[:, ct],
                op0=OP.mult,
                op1=OP.add,
            )

        # fold q scaling d^-0.5 into the q columns of w_qkvT
        nc.vector.tensor_scalar_mul(
            out=wq_sb[:, :, 0:C], in0=wq_sb[:, :, 0:C], scalar1=SCALE
        )

        # ---- QKV projections -------------------------------------------
        # q/k are stored zero-padded to 128 partitions per head (rows 64:128
        # are zeros): a K=64 matmul streams its operands at HALF the SBUF
        # bandwidth, padding the contraction to 128 restores full rate.
        # vT is padded to 128 columns per head (cols 65:128 zero) for the same
        # reason on the AV output partitions, and kept in bf16 together with
        # exp(S) so the AV matmul gets fast-weight-load and half the SBUF.
        BF16 = mybir.dt.bfloat16
        ADT = BF16 if ATTN_BF16 else MMDT
        q_pad = big.tile([P, H, NH], MMDT)
        k_pad = big.tile([P, H, N], MMDT)
        vT_pad = big.tile([P, NKC, H, P], BF16)
        # all constant-init writes go to the otherwise-idle GPSIMD engine so
        # the DVE can run the BN stats (critical path to the AllReduce) first
        nc.gpsimd.memset(vT_pad[:, :, :, D:P], 0.0)
        nc.gpsimd.memset(vT_pad[:, :, :, D : D + 1], 1.0)
        for h in range(H):
            nc.gpsimd.memset(q_pad[D:P, h, :].bitcast(F32), 0.0)
            nc.gpsimd.memset(k_pad[D:P, h, :].bitcast(F32), 0.0)
        ones_blk = small.tile([P, NKC * H], F32)
        nc.vector.memset(ones_blk, 1.0)

        for rb in range(RB):  # q (my query half only)
            for j in range(NQS):
                ps = mmpool.tile([P, QS], F32, tag="mm")
                for ct in range(CT):
                    nc.tensor.matmul(
                        out=ps,
                        lhsT=wq_sb[:, ct, rb * P : (rb + 1) * P],
                        rhs=xn_sb[:, ct, j * QS : (j + 1) * QS],
                        start=(ct == 0),
                        stop=(ct == CT - 1),
                    )
                nc.vector.tensor_copy(
                    out=q_pad[0:D, 2 * rb, j * QS : (j + 1) * QS], in_=ps[0:D, :]
                )
                nc.scalar.copy(
                    out=q_pad[0:D, 2 * rb + 1, j * QS : (j + 1) * QS],
                    in_=ps[D:P, :],
                )
        for rb in range(RB):  # k (full length)
            for j in range(N // QS):
                ps = mmpool.tile([P, QS], F32, tag="mm")
                for ct in range(CT):
                    nc.tensor.matmul(
                        out=ps,
                        lhsT=wq_sb[:, ct, C + rb * P : C + (rb + 1) * P],
                        rhs=xn_sb[:, ct, j * QS : (j + 1) * QS],
                        start=(ct == 0),
                        stop=(ct == CT - 1),
                    )
                nc.vector.tensor_copy(
                    out=k_pad[0:D, 2 * rb, j * QS : (j + 1) * QS], in_=ps[0:D, :]
                )
                nc.scalar.copy(
                    out=k_pad[0:D, 2 * rb + 1, j * QS : (j + 1) * QS],
                    in_=ps[D:P, :],
                )
        for nb in range(NKC):  # v, produced transposed: [key, (head, d)]
            ps = mmpool.tile([P, C], F32, tag="mm")
            for ct in range(CT):
                nc.tensor.matmul(
                    out=ps,
                    lhsT=xn_sb[:, ct, nb * KC : (nb + 1) * KC],
                    rhs=wq_sb[:, ct, 2 * C : 3 * C],
                    start=(ct == 0),
                    stop=(ct == CT - 1),
                )
            cp = nc.vector.tensor_copy if nb % 2 == 0 else nc.scalar.copy
            cp(
                out=vT_pad[:, nb, :, 0:D],
                in_=ps.rearrange("p (h d) -> p h d", d=D),
            )

        # ---- attention --------------------------------------------------
        if not USE_GPSIMD_BCAST:
            ones_sb = small.tile([1, D], MMDT)
            nc.vector.tensor_copy(out=ones_sb, in_=ones_blk[0:1, 0:D])
        if DUMMY_N:
            # tiny "keep-warm" operands: dummy matmuls issued right before the
            # stall points of the PE stream keep the HAM activity monitor from
            # re-throttling the PE clock to 1.2 GHz while ACT catches up.
            dum_sb = small.tile([1, DUMMY_N], ADT)
            nc.vector.tensor_copy(out=dum_sb, in_=ones_blk[0:1, 0:DUMMY_N])

        def dummy_mm():
            if DUMMY_N:
                scrap = mmpool.tile([P, QS], F32, tag="mm", name="scrap")
                nc.tensor.matmul(
                    out=scrap[0:1, 0:DUMMY_N],
                    lhsT=dum_sb[0:1, 0:1],
                    rhs=dum_sb[0:1, 0:DUMMY_N],
                    start=True,
                    stop=True,
                )

        attn_sb = big.tile([P, CT, NH], MMDT, tag="xnattn")
        out_r = out.rearrange("(rb p) n -> p rb n", p=P)

        def normalize(avp_, h_, j_):
            # rows 0:D divided by the softmax denominator in row D
            r_sb = rpool.tile([1, QS], F32, tag="r")
            nc.vector.reciprocal(out=r_sb, in_=avp_[D : D + 1, :])
            rbc = rpool.tile([D, QS], F32, tag="rbc")
            nc.gpsimd.partition_broadcast(rbc, r_sb)
            nc.vector.tensor_tensor(
                out=attn_sb[(h_ % 2) * D : (h_ % 2) * D + D, h_ // 2,
                            j_ * QS : (j_ + 1) * QS],
                in0=avp_[0:D, :],
                in1=rbc,
                op=OP.mult,
            )

        def outproj(j_):
            # output projection + bias for this query block (all heads done)
            for rb in range(RB):
                ps = mmpool.tile([P, QS], F32, tag="mm")
                for ct in range(CT):
                    nc.tensor.matmul(
                        out=ps,
                        lhsT=wo_sb[:, ct, rb * P : (rb + 1) * P],
                        rhs=attn_sb[:, ct, j_ * QS : (j_ + 1) * QS],
                        start=(ct == 0),
                        stop=(ct == CT - 1),
                    )
                o_t = opool.tile([P, QS], F32, tag="o")
                nc.vector.tensor_scalar_add(out=o_t, in0=ps, scalar1=bo_sb[:, rb])
                nc.sync.dma_start(
                    out=out_r[:, rb, j_ * QS : (j_ + 1) * QS], in_=o_t
                )

        def emit_av(p):
            # the AV matmuls run one exp-group behind the scores so exp(g-1)
            # always completes while the PE streams S(g): no per-group stall
            e_, g_, gs_, avp_, h_, j_ = p
            for u in range(gs_):
                kc = g_ * G + u
                nc.tensor.matmul(
                    out=avp_,
                    lhsT=vT_pad[:, kc, h_, :],
                    rhs=e_[:, u, :],
                    start=(kc == 0),
                    stop=(kc == NKC - 1),
                )
            if g_ == NG - 1:
                normalize(avp_, h_, j_)
                if h_ == H - 1:
                    outproj(j_)

        pend = None
        for j in range(NQS):
            for h in range(H):
                avp = mmpool.tile([P, QS], F32, tag="mm")
                for g in range(NG):
                    gs = min(G, NKC - g * G)
                    sp = spool.tile([P, G, QS], F32, tag="sp")
                    for u in range(gs):
                        kc = g * G + u
                        nc.tensor.matmul(
                            out=sp[:, u, :],
                            lhsT=k_pad[:, h, kc * KC : (kc + 1) * KC],
                            rhs=q_pad[:, h, j * QS : (j + 1) * QS],
                            start=True,
                            stop=True,
                        )
                    e_sb = epool.tile([P, G, QS], BF16, tag="e")
                    nc.scalar.activation(
                        out=e_sb[:, 0:gs, :], in_=sp[:, 0:gs, :], func=AF.Exp
                    )
                    if pend is not None:
                        emit_av(pend)
                    pend = (e_sb, g, gs, avp, h, j)
        emit_av(pend)


def build():
    nc = bacc.Bacc(
        "TRN2", target_bir_lowering=False, debug=False, num_devices=NCORES
    )
    x_mine = nc.dram_tensor("x_mine", [C, NH], MMDT, kind="ExternalInput").ap()
    x_other = nc.dram_tensor("x_other", [C, NH], MMDT, kind="ExternalInput").ap()
    w_qkvT = nc.dram_tensor("w_qkvT", [C, 3 * C], MMDT, kind="ExternalInput").ap()
    w_outT = nc.dram_tensor("w_outT", [C, C], MMDT, kind="ExternalInput").ap()
    bn_w = nc.dram_tensor("bn_w", [P, CT, 1], F32, kind="ExternalInput").ap()
    bn_b = nc.dram_tensor("bn_b", [P, CT, 1], F32, kind="ExternalInput").ap()
    b_out = nc.dram_tensor("b_out", [P, CT, 1], F32, kind="ExternalInput").ap()
    out = nc.dram_tensor("out", [C, NH], F32, kind="ExternalOutput").ap()
    with tile.TileContext(nc) as tc:
        _body(tc, x_mine, x_other, w_qkvT, w_outT, bn_w, bn_b, b_out, out)
    nc.compile()
    return nc


_nc_cache = None


def make_in_maps(x, bn_weight, bn_bias, w_qkv, w_out, b_out):
    x = np.ascontiguousarray(np.asarray(x, dtype=np.float32))
    wqT = np.ascontiguousarray(np.asarray(w_qkv, dtype=np.float32).T)
    woT = np.ascontiguousarray(np.asarray(w_out, dtype=np.float32).T)

    def vec_layout(v):
        v = np.asarray(v, dtype=np.float32)
        return np.ascontiguousarray(v.reshape(CT, P).T.reshape(P, CT, 1))

    bnw = vec_layout(bn_weight)
    bnb = vec_layout(bn_bias)
    bo = vec_layout(b_out)
    in_maps = []
    for core in range(NCORES):
        bi, half = divmod(core, 2)
        mine = np.ascontiguousarray(x[bi][:, half * NH : (half + 1) * NH])
        other = np.ascontiguousarray(x[bi][:, (1 - half) * NH : (2 - half) * NH])
        in_maps.append(
            {
                "x_mine": mine,
                "x_other": other,
                "w_qkvT": wqT,
                "w_outT": woT,
                "bn_w": bnw,
                "bn_b": bnb,
                "b_out": bo,
            }
        )
    return in_maps


def assemble(results):
    outp = np.empty((B, C, N), np.float32)
    for core in range(NCORES):
        bi, half = divmod(core, 2)
        outp[bi][:, half * NH : (half + 1) * NH] = results[core]["out"]
    return outp


def kernel(x, bn_weight, bn_bias, w_qkv, w_out, b_out):
    global _nc_cache
    if _nc_cache is None:
        _nc_cache = build()
    in_maps = make_in_maps(x, bn_weight, bn_bias, w_qkv, w_out, b_out)
    res = run_bass_kernel_spmd(_nc_cache, in_maps, list(range(NCORES)))
    return assemble(res.results)


if __name__ == "__main__":
    rng = np.random.default_rng(0)
    x = rng.standard_normal((B, C, N), dtype=np.float32)
    w_qkv = rng.standard_normal((3 * C, C), dtype=np.float32) * C**-0.5
    w_out = rng.standard_normal((C, C), dtype=np.float32) * C**-0.5
    y = kernel(
        x,
        np.ones(C, np.float32),
        np.zeros(C, np.float32),
        w_qkv,
        w_out,
        np.zeros(C, np.float32),
    )
    print(y.shape, np.abs(y).max())
